# revision 17
# baseline (speedup 1.0000x reference)
"""Distributed multi-head causal attention with RoPE on 8 TRN2 NeuronCores.

Sharding: batch (2) x head-groups (4 heads each) -> 8 cores.
  core c: batch b = c // 4, head group g = c % 4 (global heads 4g..4g+3).

v2: single software-pipelined program built for Tile's ready-heap scheduler.
Emission order doubles as priority: pass1 (V + head0 Q/K) < attention h0..h3
(foreground) < pass2 Q/K heads 1-3 (background) < output projection.  The PE
runs attention matmuls whenever they are ready and falls back to projection /
wo matmuls during softmax (ACT) stalls, so it never idles long enough to lose
the HAM clock boost.

Per-core math (bf16 matmuls, fp32 accumulate):
  - QKV in transposed layout; RoPE pair-interleave folded into a host-side
    row permutation; 1/sqrt(hd) folded into wq.  RoPE eviction: ACT copies
    PSUM->SBUF bf16 (plain + half-swapped), then 3 bf16 DVE ops.
  - Scores ST[k,q] = K_blk.T @ Q chunk; exp on ACT in EB=2 batches; partial
    causal blocks masked by 0/1 pattern multiply (bf16).  Softmax denominator:
    bf16 pair-sum tree (1 DVE add per 2 k-tiles) feeding a PSUM-accumulated
    ones-matmul; reciprocal applied to the PV output.
  - Per-head half-sequence AllGather (bf16) across the 4 cores of the batch
    group, firing as soon as each half of a head is normalized.
  - Output projection column-sharded, split in two stages (heads 0/1 then
    2/3) so the last head's AllGather hides behind stage-A matmuls.
"""

import math

import numpy as np
import ml_dtypes

BSZ, SEQ, DIM, NH, HD = 2, 2048, 2048, 16, 128
NCORES = 8
GSIZE = 4            # cores per batch group
HPC = NH // GSIZE    # heads per core = 4
DLOC = HPC * HD      # local head dims = 512
QC = 512             # q-chunk (matmul moving free dim)
NQC = SEQ // QC      # 4
KT = 128             # k-tile
NKT = SEQ // KT      # 16
IC = 128             # contraction tile
NIC = DIM // IC      # 16
EB = 2               # k-tiles per exp batch
BF16 = ml_dtypes.bfloat16


def _build_and_compile(block_plan_key, n_pat):
    """Build + compile the SPMD bass graph.  block_plan_key is a tuple over
    q-chunks of tuples of (kt, pat_idx or -1, q0)."""
    import concourse.bass as bass
    import concourse.tile as tile
    from concourse import bacc, mybir
    from contextlib import ExitStack

    f32 = mybir.dt.float32
    bf16 = mybir.dt.bfloat16
    ts = bass.ts

    # NOTE: matmul PSUM writes must start at the bank base, so all score/PV/
    # colsum matmuls run full q-chunk width; q0 is unused here.
    block_plan = [[(kt, (None if p < 0 else p)) for kt, p, _ in qcp]
                  for qcp in block_plan_key]

    nc = bacc.Bacc("TRN2", target_bir_lowering=False, debug=False,
                   num_devices=NCORES)

    xT_d = nc.dram_tensor("xT", [128, NQC, NIC, QC], bf16,
                          kind="ExternalInput").ap()
    wq0_d = nc.dram_tensor("wq0T", [128, NIC, HD], bf16,
                           kind="ExternalInput").ap()
    wk0_d = nc.dram_tensor("wk0T", [128, NIC, HD], bf16,
                           kind="ExternalInput").ap()
    wqR_d = nc.dram_tensor("wqRT", [128, NIC, DLOC - HD], bf16,
                           kind="ExternalInput").ap()
    wkR_d = nc.dram_tensor("wkRT", [128, NIC, DLOC - HD], bf16,
                           kind="ExternalInput").ap()
    wvT_d = nc.dram_tensor("wvT", [128, NIC, DLOC], bf16,
                           kind="ExternalInput").ap()
    woT_d = nc.dram_tensor("woT", [128, NIC, DLOC], bf16,
                           kind="ExternalInput").ap()
    cos2_d = nc.dram_tensor("cos2", [HD, SEQ], bf16, kind="ExternalInput").ap()
    sinpm_d = nc.dram_tensor("sinpm", [HD, SEQ], bf16,
                             kind="ExternalInput").ap()
    pat_d = nc.dram_tensor("pat", [128, max(n_pat, 1), QC], bf16,
                           kind="ExternalInput").ap()
    out_d = nc.dram_tensor("out", [SEQ, DLOC], bf16, kind="ExternalOutput").ap()

    groups = [[0, 1, 2, 3], [4, 5, 6, 7]]
    Exp = mybir.ActivationFunctionType.Exp

    with tile.TileContext(nc) as tc, ExitStack() as top:
        persist = top.enter_context(tc.tile_pool(name="persist", bufs=1))
        atpool = top.enter_context(tc.tile_pool(name="atpool", bufs=3))
        xpool = top.enter_context(tc.tile_pool(name="xpool", bufs=8))
        ptpool = top.enter_context(tc.tile_pool(name="ptpool", bufs=6))
        ropep = top.enter_context(tc.tile_pool(name="ropep", bufs=2))
        s2pool = top.enter_context(tc.tile_pool(name="s2pool", bufs=4))
        rbpool = top.enter_context(tc.tile_pool(name="rbpool", bufs=2))
        ltpool = top.enter_context(tc.tile_pool(name="ltpool", bufs=4))
        otpool = top.enter_context(tc.tile_pool(name="otpool", bufs=2))
        dram = top.enter_context(
            tc.tile_pool(name="dram", bufs=2 * HPC, space="DRAM"))
        ps_st = top.enter_context(
            tc.tile_pool(name="ps_st", bufs=2, space="PSUM"))
        ps_pv = top.enter_context(
            tc.tile_pool(name="ps_pv", bufs=1, space="PSUM"))
        # cs (softmax colsum) shares the accumulator pool: pass1 gets a
        # 3-deep projection rotation while attention holds one slot per qc
        # for the colsum.
        ps_ac = top.enter_context(
            tc.tile_pool(name="ps_ac", bufs=3, space="PSUM"))

        qt_sb = persist.tile([128, HPC, SEQ], bf16, name="qt_sb")
        kt_sb = persist.tile([128, HPC, SEQ], bf16, name="kt_sb")
        v_sb = persist.tile([128, NKT, DLOC], bf16, name="v_sb")
        ones_sb = persist.tile([128, 128], bf16, name="ones_sb")
        pat_sb = persist.tile([128, max(n_pat, 1), QC], bf16, name="pat_sb")
        cos2_sb = persist.tile([HD, SEQ], bf16, name="cos2_sb")
        sinpm_sb = persist.tile([HD, SEQ], bf16, name="sinpm_sb")
        wq0_sb = persist.tile([128, NIC, HD], bf16, name="wq0_sb")
        wk0_sb = persist.tile([128, NIC, HD], bf16, name="wk0_sb")
        wqR_sb = persist.tile([128, NIC, DLOC - HD], bf16, name="wqR_sb")
        wkR_sb = persist.tile([128, NIC, DLOC - HD], bf16, name="wkR_sb")
        wv_sb = persist.tile([128, NIC, DLOC], bf16, name="wv_sb")
        wo_sb = persist.tile([128, NIC, DLOC], bf16, name="wo_sb")
        po_sb = persist.tile([128, NKT, DLOC], bf16, name="po_sb")

        nc.vector.memset(ones_sb[:], 1.0)
        # warm up the ACT exp table-set before any copies/exps are needed
        warm = s2pool.tile([128, QC], bf16, name="warm", tag="s2")
        nc.vector.memset(warm[:, 0:8], 0.0)
        nc.scalar.activation(warm[:, 0:8], warm[:, 0:8], Exp)

        # ---- startup DMAs: weights/consts on gpsimd (idle until first AG),
        # x tiles on sync, so the first matmul's operands land in parallel ----
        for qq in range(4):
            sl = bass.ds(qq * (NIC // 4), NIC // 4)
            nc.gpsimd.dma_start(out=wq0_sb[:, sl, :], in_=wq0_d[:, sl, :])
            nc.gpsimd.dma_start(out=wk0_sb[:, sl, :], in_=wk0_d[:, sl, :])
        nc.gpsimd.dma_start(out=cos2_sb[:], in_=cos2_d[:, :])
        nc.gpsimd.dma_start(out=sinpm_sb[:], in_=sinpm_d[:, :])
        nc.gpsimd.dma_start(out=pat_sb[:], in_=pat_d[:, :, :])

        XG = 2                      # x chunks per DMA group

        def load_x(qc, deep=False):
            # deep=True (pass1 only): route the first two groups through the
            # then-idle ltpool so the chunk boundary has 2-chunk prefetch
            # depth for the groups consumed first.
            tiles = []
            for g in range(NIC // XG):
                pool = ltpool if (deep and g < 2) else xpool
                t = pool.tile([128, XG, QC], bf16, name="x_sb",
                              tag="lt" if (deep and g < 2) else "x_sb")
                nc.sync.dma_start(
                    out=t[:], in_=xT_d[:, qc, bass.ds(g * XG, XG), :])
                tiles.append(t)
            return tiles

        def x_at(xs, ic):
            return xs[ic // XG][:, ic % XG, :]

        def rope_evict(acc, dst_slice, qc):
            """PSUM fp32 acc -> RoPE'd bf16 dst.  ACT does the PSUM reads
            (plain + half-swap), DVE does 3 bf16 ops."""
            qb = ropep.tile([128, QC], bf16, name="qb", tag="qb")
            qs = ropep.tile([128, QC], bf16, name="qs", tag="qs")
            nc.scalar.copy(qb[:], acc[:])
            nc.vector.tensor_copy(qs[0:64, :], qb[64:128, :])
            nc.vector.tensor_copy(qs[64:128, :], qb[0:64, :])
            m1 = ropep.tile([128, QC], bf16, name="m1", tag="m1")
            sw = ropep.tile([128, QC], bf16, name="sw", tag="sw")
            nc.vector.tensor_mul(m1[:], qb[:], cos2_sb[:, ts(qc, QC)])
            nc.vector.tensor_mul(sw[:], qs[:], sinpm_sb[:, ts(qc, QC)])
            nc.vector.tensor_add(dst_slice, m1[:], sw[:])

        def proj_qk(w_sb, hlo, dst, h, qc, xs):
            acc = ps_ac.tile([128, QC], f32, name="pacc", tag="pacc")
            for ic in range(NIC):
                nc.tensor.matmul(acc[:], w_sb[:, ic, bass.ds(hlo, HD)],
                                 x_at(xs, ic),
                                 start=(ic == 0), stop=(ic == NIC - 1))
            rope_evict(acc, dst[:, h, ts(qc, QC)], qc)

        # ---------------- attention (foreground) ----------------
        ag_outs = {}   # (head, half) -> gathered DRAM buffer
        at_cur = {}    # h -> live at tile

        def attn_head(h, qcs=None):
            for qc in (range(NQC) if qcs is None else qcs):
                kts = block_plan[qc]
                nkt = len(kts)
                npair = nkt // EB
                if qc % 2 == 0:
                    at_cur[h] = atpool.tile([128, SEQ // 2], bf16, name="at",
                                            tag="at")
                at = at_cur[h]
                pv = ps_pv.tile([128, QC], f32, name="pv", tag="pv")
                cs = ps_ac.tile([128, QC], f32, name="cs", tag="pacc")
                for pi in range(npair):
                    pair = kts[pi * EB:(pi + 1) * EB]
                    st = ps_st.tile([128, EB, QC], f32, name="st", tag="st")
                    for j, (kt, _) in enumerate(pair):
                        nc.tensor.matmul(
                            st[:, j, :],
                            kt_sb[:, h, ts(kt, KT)],
                            qt_sb[:, h, ts(qc, QC)],
                            start=True, stop=True)
                    pt = ptpool.tile([128, EB, QC], bf16, name="pt", tag="pt")
                    nc.scalar.activation(pt[:], st[:], Exp)
                    for j, (kt, pidx) in enumerate(pair):
                        if pidx is not None:
                            nc.vector.tensor_mul(
                                pt[:, j, :], pt[:, j, :], pat_sb[:, pidx, :])
                    for j, (kt, _) in enumerate(pair):
                        i = pi * EB + j
                        nc.tensor.matmul(
                            pv[:], v_sb[:, kt, ts(h, HD)], pt[:, j, :],
                            start=(i == 0), stop=(i == nkt - 1))
                    s2 = s2pool.tile([128, QC], bf16, name="s2", tag="s2")
                    nc.vector.tensor_add(s2[:], pt[:, 0, :], pt[:, 1, :])
                    if pi % 2 == 0:
                        s2_prev = s2
                    else:
                        s4 = s2pool.tile([128, QC], bf16, name="s4", tag="s4")
                        nc.vector.tensor_add(s4[:], s2_prev[:], s2[:])
                        nc.tensor.matmul(cs[:], ones_sb[:], s4[:],
                                         start=(pi == 1), stop=(pi == npair - 1))
                rb = rbpool.tile([128, QC], f32, name="rb", tag="rb")
                nc.vector.reciprocal_approx_fast(rb[:], cs[:])
                nc.vector.tensor_mul(at[:, ts(qc % 2, QC)], pv[:], rb[:])
                if qc % 2 == 1:
                    half = qc // 2
                    ag_in = dram.tile([128, SEQ // 2], bf16, name="ag_in",
                                      tag="ag_in")
                    ag_out = dram.tile([GSIZE * 128, SEQ // 2], bf16,
                                       name="ag_out", tag=f"ag_out{h}_{half}")
                    nc.gpsimd.dma_start(out=ag_in[:], in_=at[:])
                    nc.gpsimd.collective_compute(
                        "AllGather", mybir.AluOpType.bypass,
                        replica_groups=groups,
                        ins=[ag_in[:].opt()],
                        outs=[ag_out[:].opt()])
                    ag_outs[(h, half)] = ag_out

        # ---------------- pass 1: head-0 Q/K + all V ----------------
        x_sb = {0: load_x(0, deep=True)}
        for g in range(NIC // XG):
            nc.gpsimd.dma_start(out=wv_sb[:, bass.ds(g * XG, XG), :],
                                in_=wvT_d[:, bass.ds(g * XG, XG), :])
        for qc in range(NQC):
            if qc + 1 < NQC:
                x_sb[qc + 1] = load_x(qc + 1, deep=True)
            xs = x_sb[qc]
            proj_qk(wq0_sb, 0, qt_sb, 0, qc, xs)
            proj_qk(wk0_sb, 0, kt_sb, 0, qc, xs)
            for sl in range(QC // 128):
                s = qc * (QC // 128) + sl
                acc = ps_ac.tile([128, DLOC], f32, name="pacc", tag="pacc")
                for ic in range(NIC):
                    nc.tensor.matmul(
                        acc[:], x_at(xs, ic)[:, ts(sl, 128)],
                        wv_sb[:, ic, :],
                        start=(ic == 0), stop=(ic == NIC - 1))
                nc.scalar.copy(v_sb[:, s, :], acc[:])

        # wqR/wkR for pass 2 (sync queue, lands during pass1/attn0)
        for qq in range(8):
            sl = bass.ds(qq * (NIC // 8), NIC // 8)
            nc.gpsimd.dma_start(out=wqR_sb[:, sl, :], in_=wqR_d[:, sl, :])
            nc.gpsimd.dma_start(out=wkR_sb[:, sl, :], in_=wkR_d[:, sl, :])
        # wo preload (needed by WO stage A, mid-attention)
        for qq in range(8):
            sl = bass.ds(qq * (NIC // 8), NIC // 8)
            nc.gpsimd.dma_start(out=wo_sb[:, sl, :], in_=woT_d[:, sl, :])

        attn_head(0)

        # pass2-h (producer) emitted before attn-h (consumer); during attn-h
        # the next head's projections are the lower-priority PE filler.
        for h in range(1, HPC):
            for qc in range(NQC):
                xs = load_x(qc)
                proj_qk(wqR_sb, (h - 1) * HD, qt_sb, h, qc, xs)
                proj_qk(wkR_sb, (h - 1) * HD, kt_sb, h, qc, xs)
            attn_head(h)

        # ---------------- output projection (lowest priority) ----------------
        SH = NKT // 2   # s-tiles per AG half
        lt_cache = {}

        def load_lt(h, s):
            """Gathered attnT [128, GSIZE, 128] for (local head h, s-tile);
            two s-tiles per DMA.  Sync queue: emitted in AG-completion order."""
            key = (h, s // 2)
            if key not in lt_cache:
                lt = ltpool.tile([128, GSIZE, 256], bf16, name="lt", tag="lt")
                gsrc = ag_outs[(h, s // SH)].rearrange("(r p) s -> p r s",
                                                       p=128)
                nc.sync.dma_start(
                    out=lt[:], in_=gsrc[:, :, ts((s % SH) // 2, 256)])
                lt_cache[key] = lt
            return lt_cache[key][:, :, ts(s % 2, 128)]

        # stage A: heads 0-1 (ready early; fills attn-h3 stalls)
        for s in range(NKT):
            acc = ps_ac.tile([128, DLOC], f32, name="pacc", tag="pacc")
            for h in range(2):
                lt = load_lt(h, s)
                for r in range(GSIZE):
                    nc.tensor.matmul(
                        acc[:], lt[:, r, :], wo_sb[:, GSIZE * r + h, :],
                        start=((h, r) == (0, 0)),
                        stop=((h, r) == (1, GSIZE - 1)))
            nc.scalar.copy(po_sb[:, s, :], acc[:])
        # stage A2: head 2 (fills the last head's AllGather window)
        for s in range(NKT):
            acc = ps_ac.tile([128, DLOC], f32, name="pacc", tag="pacc")
            lt = load_lt(2, s)
            for r in range(GSIZE):
                nc.tensor.matmul(
                    acc[:], lt[:, r, :], wo_sb[:, GSIZE * r + 2, :],
                    start=(r == 0), stop=(r == GSIZE - 1))
            nc.vector.tensor_add(po_sb[:, s, :], acc[:], po_sb[:, s, :])

        def stage_b(srange):
            for s in srange:
                acc = ps_ac.tile([128, DLOC], f32, name="pacc", tag="pacc")
                lt = load_lt(HPC - 1, s)
                for r in range(GSIZE):
                    nc.tensor.matmul(
                        acc[:], lt[:, r, :], wo_sb[:, GSIZE * r + HPC - 1, :],
                        start=(r == 0), stop=(r == GSIZE - 1))
                ot = otpool.tile([128, DLOC], bf16, name="ot", tag="ot")
                nc.vector.tensor_add(ot[:], acc[:], po_sb[:, s, :])
                nc.sync.dma_start(out=out_d[ts(s, 128), :], in_=ot[:])

        # stage B: head 3, split by AllGather half
        stage_b(range(NKT // 2))
        stage_b(range(NKT // 2, NKT))

    nc.compile()
    return nc


_CACHE = {}


def _get_compiled(block_plan_key, n_pat):
    key = (block_plan_key, n_pat)
    if key not in _CACHE:
        _CACHE[key] = _build_and_compile(block_plan_key, n_pat)
    return _CACHE[key]


def _plan_from_mask(mask):
    """Derive per-q-chunk k-tile lists + dedup'd 0/1 patterns from the mask.

    Plan entries are (kt, pat_idx or -1, q0): the leading q0 columns of the
    q-chunk are fully masked; a [KT, QC] 0/1 pattern multiplies the block.
    """
    keep = mask > -1e20
    if not np.all(mask[keep] == 0.0):
        raise NotImplementedError("only 0/-inf style masks supported")
    pats = []
    pat_index = {}
    plan = []
    for qc in range(NQC):
        qs = slice(qc * QC, (qc + 1) * QC)
        row = []
        for kt in range(NKT):
            ks = slice(kt * KT, (kt + 1) * KT)
            blk = keep[qs, ks]            # [QC, KT]
            if not blk.any():
                continue
            if blk.all():
                row.append((kt, -1, 0))
                continue
            p = np.ascontiguousarray(blk.T).astype(np.float32)  # [KT, QC]
            kb = p.tobytes()
            if kb not in pat_index:
                pat_index[kb] = len(pats)
                pats.append(p)
            col_any = blk.any(axis=1)
            q0 = int(np.argmax(col_any)) if col_any.any() else 0
            if col_any[:q0].any():
                q0 = 0
            row.append((kt, pat_index[kb], q0))
        plan.append(tuple(row))
    return tuple(plan), pats


def _head_perm():
    """Row permutation per head: even dims first, then odd."""
    perm = []
    for h in range(NH):
        base = h * HD
        perm.extend(base + np.arange(0, HD, 2))
        perm.extend(base + np.arange(1, HD, 2))
    return np.array(perm)


def _pmajor(wT, lo=0, hi=None):
    """[DIM, D] (already transposed weight) -> [128, NIC, hi-lo] partition-
    major layout: out[p, c, d] = wT[c*128 + p, lo + d]."""
    hi = wT.shape[1] if hi is None else hi
    return np.ascontiguousarray(
        wT[:, lo:hi].reshape(NIC, 128, hi - lo).transpose(1, 0, 2)
    ).astype(BF16)


def _prep_in_maps(x, wq, wk, wv, wo, freqs_cos, freqs_sin, pats, n_pat):
    perm = _head_perm()
    wq_p = (wq / math.sqrt(HD))[perm]
    wk_p = wk[perm]

    cosT = np.ascontiguousarray(freqs_cos.T)        # [64, SEQ]
    sinT = np.ascontiguousarray(freqs_sin.T)
    cos2 = np.concatenate([cosT, cosT], axis=0).astype(BF16)   # [128, SEQ]
    sinpm = np.concatenate([-sinT, sinT], axis=0).astype(BF16)

    if n_pat:
        pat_np = np.stack(pats)                     # [n_pat, KT, QC]
    else:
        pat_np = np.zeros((1, KT, QC), dtype=np.float32)
    pat_h = np.ascontiguousarray(pat_np.transpose(1, 0, 2)).astype(BF16)

    xh = []
    for b in range(BSZ):
        xT = x[b].T.reshape(NIC, 128, NQC, QC)
        xh.append(np.ascontiguousarray(
            xT.transpose(1, 2, 0, 3)).astype(BF16))

    in_maps = []
    for c in range(NCORES):
        b, g = c // GSIZE, c % GSIZE
        rows = slice(g * DLOC, (g + 1) * DLOC)
        wqT = wq_p[rows].T   # [DIM, DLOC]
        wkT = wk_p[rows].T
        in_maps.append({
            "xT": xh[b],
            "wq0T": _pmajor(wqT, 0, HD),
            "wqRT": _pmajor(wqT, HD, DLOC),
            "wk0T": _pmajor(wkT, 0, HD),
            "wkRT": _pmajor(wkT, HD, DLOC),
            "wvT": _pmajor(wv[rows].T),
            "woT": _pmajor(wo[rows].T),
            "cos2": cos2,
            "sinpm": sinpm,
            "pat": pat_h,
        })
    return in_maps


def kernel(x, wq, wk, wv, wo, freqs_cos, freqs_sin, mask):
    x = np.asarray(x, dtype=np.float32)
    wq = np.asarray(wq, dtype=np.float32)
    wk = np.asarray(wk, dtype=np.float32)
    wv = np.asarray(wv, dtype=np.float32)
    wo = np.asarray(wo, dtype=np.float32)
    freqs_cos = np.asarray(freqs_cos, dtype=np.float32)
    freqs_sin = np.asarray(freqs_sin, dtype=np.float32)
    mask = np.asarray(mask, dtype=np.float32)

    plan, pats = _plan_from_mask(mask)
    n_pat = len(pats)
    nc = _get_compiled(plan, n_pat)

    in_maps = _prep_in_maps(x, wq, wk, wv, wo, freqs_cos, freqs_sin,
                            pats, n_pat)

    from concourse.bass_utils import run_bass_kernel_spmd
    res = run_bass_kernel_spmd(nc, in_maps, core_ids=list(range(NCORES)))
    outs = res.results

    full = np.empty((BSZ, SEQ, DIM), dtype=np.float32)
    for c in range(NCORES):
        b, g = c // GSIZE, c % GSIZE
        full[b][:, g * DLOC:(g + 1) * DLOC] = outs[c]["out"]
    return full
